# revision 34
# baseline (speedup 1.0000x reference)
"""Multi-head attention (B=4, T=S=2048, E=1024, H=16, D=64) on 8 TRN2 NeuronCores.

Sharding: core c handles batch b=c//2 and head-group g=c%2 (8 of 16 heads).
Each core computes its 8 heads' attention plus the matching column-slice of
the output projection, producing a partial [T, E] bf16 output. Host sums the
two partials per batch and adds bo.

On-chip dataflow (all matmuls bf16 with fp32 PSUM accumulation):
  qT[d,t] = WqT.T @ queryT       (d-major projections, per 128-dim head pair)
  kT[d,t] likewise; v[s,d] natural via value.T as the stationary operand
  S.T[s,t] = kT_h.T @ qT_h       (two heads row-packed in the 128-row PE array)
  expS.T   = exp(S.T * 1/8)      (ScalarE, PSUM -> SBUF bf16)
  O[t,d],den = expS.T.T @ [v_h|1]  (t-major PV: full 128 output partitions,
                                    65-column moving operand -> half the PE
                                    column-cycles of the d-major layout)
  OnormB   = O * (1/den)         (DVE per-partition scalar multiply)
  OnormT   = transpose(OnormB)   (PE transpose via identity, [t,dc]->[dc,t])
  partial  = OnormT.T @ WoSlice  (accumulate over the core's 4 head pairs)

Emission is software-pipelined: stage s=(pair, t-quarter) in pair-major
order; each stage's 16 score-tile slots interleave the previous stage's PV
accumulation (t-subtiles 0,1 during slots 0-7; 2,3 during slots 8-15, so
normalization staggers) plus spread-out projection / v-projection /
out-projection work, keeping ScalarE (the exp bottleneck) continuously fed.
"""

from contextlib import ExitStack

import numpy as np
import ml_dtypes

B, T, S, E = 4, 2048, 2048, 1024
H, D = 16, 64
DC = 512          # dims per core (8 heads x 64)
NP = 4            # head pairs per core
NS = S // 128     # 16 s-tiles
NQ = 4            # t-quarters of 512

_BF16 = ml_dtypes.bfloat16

_cached = None


def _build(repeats=1):
    import concourse.bass as bass
    import concourse.mybir as mybir
    import concourse.tile as tile
    from concourse import bacc

    f32 = mybir.dt.float32
    bf16 = mybir.dt.bfloat16
    AF = mybir.ActivationFunctionType

    nc = bacc.Bacc("TRN2", target_bir_lowering=False)

    qT_d = nc.dram_tensor("qT", [E, T], bf16, kind="ExternalInput")
    kT_d = nc.dram_tensor("kT", [E, S], bf16, kind="ExternalInput")
    vT_d = nc.dram_tensor("vT", [E, S], bf16, kind="ExternalInput")
    WqT_d = nc.dram_tensor("WqT", [E, DC], bf16, kind="ExternalInput")
    WkT_d = nc.dram_tensor("WkT", [E, DC], bf16, kind="ExternalInput")
    WvT_d = nc.dram_tensor("WvT", [E, DC], bf16, kind="ExternalInput")
    WoS_d = nc.dram_tensor("WoS", [DC, E], bf16, kind="ExternalInput")
    bq_d = nc.dram_tensor("bq", [128, NP], f32, kind="ExternalInput")
    bk_d = nc.dram_tensor("bk", [128, NP], f32, kind="ExternalInput")
    bv_d = nc.dram_tensor("bv", [1, DC], f32, kind="ExternalInput")
    out_d = nc.dram_tensor("out", [T, E], bf16, kind="ExternalOutput")

    with tile.TileContext(nc) as tc, ExitStack() as ctx:
        persist = ctx.enter_context(tc.tile_pool(name="persist", bufs=1))
        psc = ctx.enter_context(tc.tile_pool(name="psc", bufs=2, space="PSUM"))
        ppv = ctx.enter_context(tc.tile_pool(name="ppv", bufs=2, space="PSUM"))
        pmx = ctx.enter_context(tc.tile_pool(name="pmx", bufs=2, space="PSUM"))
        expool = ctx.enter_context(tc.tile_pool(name="expool", bufs=26))
        small = ctx.enter_context(tc.tile_pool(name="small", bufs=3))
        ocp_pool = ctx.enter_context(tc.tile_pool(name="ocp", bufs=3))
        xin = ctx.enter_context(tc.tile_pool(name="xin", bufs=14))
        wpool = ctx.enter_context(tc.tile_pool(name="wts", bufs=24))

        # ---- persistent SBUF tiles ----
        qTs = [persist.tile([128, T], bf16, tag=f"qT{p}", name=f"qT{p}") for p in range(NP)]
        kTs = [persist.tile([128, S], bf16, tag=f"kT{p}", name=f"kT{p}") for p in range(NP)]
        vaug = [persist.tile([128, 8 * 65], bf16, tag=f"va{st}", name=f"va{st}") for st in range(NS)]
        WoSs = [persist.tile([128, E], bf16, tag=f"wo{p}", name=f"wo{p}") for p in range(NP)]
        # transposed normalized O blocks: OnT[p][tt] is [dc=128, t=128] bf16
        OnT = [[persist.tile([128, 128], bf16, tag=f"ot{p}_{t}", name=f"ot{p}_{t}")
                for t in range(16)] for p in range(NP)]
        bq_sb = persist.tile([128, NP], f32, tag="bq", name="bq_sb")
        bk_sb = persist.tile([128, NP], f32, tag="bk", name="bk_sb")
        bv_sb = persist.tile([128, DC], f32, tag="bv", name="bv_sb")

        # constants ride the (otherwise idle) GPSIMD software-DGE queue so
        # the two hardware queues stay clear for the streaming inputs; the
        # fatter ones (wv tiles, WoS, identity) are emitted inside the main
        # block after the v weights, ordered by first use
        nc.gpsimd.dma_start(out=bq_sb, in_=bq_d[:, :])
        nc.gpsimd.dma_start(out=bk_sb, in_=bk_d[:, :])

        def load_consts():
            bv_ap = bv_d[:, :]
            bv_bcast_ap = bass.AP(
                tensor=bv_ap.tensor,
                offset=bv_ap.offset,
                ap=[[0, 128], bv_ap.ap[-1]],
            )
            nc.gpsimd.dma_start(out=bv_sb, in_=bv_bcast_ap)
            for p in range(NP):
                nc.gpsimd.dma_start(out=WoSs[p], in_=WoS_d[p * 128:(p + 1) * 128, :])
        for st in range(NS):
            va3 = vaug[st].rearrange("p (h x) -> p h x", x=65)
            nc.vector.memset(va3[:, :, 64:65], 1.0)

        def load_wtiles(dram, deng):
            ts_ = []
            for e in range(8):
                t_ = wpool.tile([128, DC], bf16, tag="w", name="wt")
                deng.dma_start(out=t_, in_=dram[e * 128:(e + 1) * 128, :])
                ts_.append(t_)
            return ts_

        def proj_thunks(p, x_dram, w_tiles, dst, bias_sb, deng, xtag, halves=(0, 1),
                        preload=False, use_sc=False):
            """One pair's q/k projection as a thunk list: two column-halves;
            per half, stream 8 e-tile chunks (load + 2 quarter-MMs each),
            then bias-add the two finished quarters out of PSUM. With
            preload=True all 8 DMAs are issued before the first matmul;
            use_sc puts the psum in the (still idle) scores ring so the two
            startup halves can interleave without exhausting the mx slots."""
            thunks = []
            for half in halves:
                ps_pair = []  # the two quarter psums of this half (alloc lazily)
                xh = []

                def load_chunk(e, half, xh):
                    xt = xin.tile([128, 1024], bf16, tag=xtag,
                                  name="xin", bufs=6)
                    deng.dma_start(
                        out=xt,
                        in_=x_dram[e * 128:(e + 1) * 128,
                                   half * 1024:(half + 1) * 1024])
                    xh.append(xt)

                def open_half(half=half, ps_pair=ps_pair, xh=xh):
                    if use_sc:
                        ps = psc.tile([128, 1024], f32, tag="sc", name="pj_ps")
                        ps_pair.extend([ps[:, 0:512], ps[:, 512:1024]])
                    else:
                        for qi in range(2):
                            ps_pair.append(pmx.tile([128, 512], f32, tag="mx", name="mx_ps"))
                    # issue all DMAs up front (preload) or keep a 2-chunk
                    # prefetch lead so a matmul never waits the ~2.3us
                    # issue-to-data DMA latency
                    for e in range(8 if preload else 2):
                        load_chunk(e, half, xh)

                thunks.append(open_half)

                for e in range(8):
                    def echunk(e=e, half=half, ps_pair=ps_pair, xh=xh):
                        if not preload and e < 6:
                            load_chunk(e + 2, half, xh)
                        xt = xh[e]
                        for qi in range(2):
                            nc.tensor.matmul(
                                ps_pair[qi],
                                w_tiles[e][:, p * 128:(p + 1) * 128],
                                xt[:, qi * 512:(qi + 1) * 512],
                                start=(e == 0),
                                stop=(e == 7),
                            )
                    thunks.append(echunk)

                def close_half(half=half, ps_pair=ps_pair):
                    for qi in range(2):
                        q = half * 2 + qi
                        nc.vector.tensor_scalar_add(
                            dst[:, q * 512:(q + 1) * 512],
                            ps_pair[qi], bias_sb[:, p:p + 1])
                    ps_pair.clear()

                thunks.append(close_half)
            return thunks

        def vproj_thunks(wv_tiles, dh, dengs):
            """V projection for head-quad dh (4 heads, N=256), streamed in
            two s-halves. dh=0 feeds pairs 0-1 (needed by stage 1); dh=1
            feeds pairs 2-3 (needed from stage 9) and can spread late."""
            thunks = []
            for half in range(2):
                vh = []

                def load_half(half=half, vh=vh):
                    for e in range(8):
                        vt = xin.tile([128, 1024], bf16, tag="xv",
                                      name="xin", bufs=10)
                        dengs[half].dma_start(
                            out=vt,
                            in_=vT_d[e * 128:(e + 1) * 128,
                                     half * 1024:(half + 1) * 1024])
                        vh.append(vt)

                thunks.append(load_half)
                for sti in range(8):
                    holder = []

                    def vst_a(sti=sti, half=half, vh=vh, holder=holder):
                        # the "pv" slots now belong to the chain accumulators
                        # (strict ring order); stage 0's units ride the "sc"
                        # ring (scores psum has slack there), the rest "mx"
                        if dh == 0 and half == 0:
                            ps = psc.tile([128, 512], f32, tag="sc", name="vp_ps")
                        else:
                            ps = pmx.tile([128, 512], f32, tag="mx", name="mx_ps")
                        holder.append(ps)
                        for e in range(4):
                            nc.tensor.matmul(
                                ps[:, 0:256],
                                vh[e][:, sti * 128:(sti + 1) * 128],
                                wv_tiles[e][:, dh * 256:(dh + 1) * 256],
                                start=(e == 0),
                                stop=False,
                            )

                    def vst_b(sti=sti, half=half, vh=vh, holder=holder):
                        st = half * 8 + sti
                        ps = holder.pop()
                        for e in range(4, 8):
                            nc.tensor.matmul(
                                ps[:, 0:256],
                                vh[e][:, sti * 128:(sti + 1) * 128],
                                wv_tiles[e][:, dh * 256:(dh + 1) * 256],
                                start=False,
                                stop=(e == 7),
                            )
                        va3 = vaug[st].rearrange("p (h x) -> p h x", x=65)
                        nc.vector.tensor_add(
                            va3[:, dh * 4:(dh + 1) * 4, 0:64],
                            ps[:, 0:256].rearrange("p (h x) -> p h x", x=64),
                            bv_sb[:, dh * 256:(dh + 1) * 256].rearrange(
                                "p (h x) -> p h x", x=64),
                        )
                        if half == 0 and sti == 7:
                            vh.clear()
                    thunks.append(vst_a)
                    thunks.append(vst_b)
            return thunks

        def outproj_thunks(tq):
            """Each (tt, c) unit as two ~1k-cycle thunks (pairs 0-1 matmuls,
            then pairs 2-3 + copy-out) so the per-slot PE load stays smooth;
            output DMAs alternate between the two hardware queues."""
            thunks = []
            for ui, (tt, c) in enumerate([(tt, c)
                                          for tt in range(tq * 4, tq * 4 + 4)
                                          for c in range(2)]):
                holder = []

                def unit_a(tt=tt, c=c, holder=holder):
                    op_ps = pmx.tile([128, 512], f32, tag="mx", name="mx_ps")
                    holder.append(op_ps)
                    for p in range(2):
                        nc.tensor.matmul(
                            op_ps,
                            OnT[p][tt],
                            WoSs[p][:, c * 512:(c + 1) * 512],
                            start=(p == 0),
                            stop=False,
                        )

                def unit_b(tt=tt, c=c, holder=holder,
                           deng=nc.sync):
                    op_ps = holder.pop()
                    for p in range(2, NP):
                        nc.tensor.matmul(
                            op_ps,
                            OnT[p][tt],
                            WoSs[p][:, c * 512:(c + 1) * 512],
                            start=False,
                            stop=(p == 3),
                        )
                    oc = ocp_pool.tile([128, 512], bf16, tag="ocp", name="oc")
                    nc.vector.tensor_copy(oc, op_ps)
                    deng.dma_start(
                        out=out_d[tt * 128:(tt + 1) * 128,
                                  c * 512:(c + 1) * 512],
                        in_=oc)
                thunks.append(unit_a)
                thunks.append(unit_b)
            return thunks

        def outproj_tail_unit(tt):
            # tail variant: "sc" psum tiles are free once scoring has ended,
            # so use wide [128,1024] units to avoid mx-slot serialization
            op_ps = psc.tile([128, 1024], f32, tag="sc", name="sc_ps")
            for c in range(2):
                for p in range(NP):
                    nc.tensor.matmul(
                        op_ps[:, c * 512:(c + 1) * 512],
                        OnT[p][tt],
                        WoSs[p][:, c * 512:(c + 1) * 512],
                        start=(p == 0),
                        stop=(p == 3),
                    )
            oc = ocp_pool.tile([128, 1024], bf16, tag="ocpw", name="ocw")
            nc.vector.tensor_copy(oc, op_ps)
            (nc.gpsimd if tt >= 14 else nc.sync).dma_start(
                out=out_d[tt * 128:(tt + 1) * 128, :], in_=oc)

        class PrevStage:
            def __init__(self, p, tq, exs, pv0):
                self.p, self.tq, self.exs = p, tq, exs
                self.pv = [pv0, None]  # [128,260] psums: halves (tt 0,1 | 2,3)

        # (onb_tile, p, tt) records awaiting PE transpose + copy-out
        pending_tr = []

        def alloc_pv():
            """Chain accumulator: four start-less accumulation groups share
            the bank (a matmul `start` would pending-zero the whole 2KB
            region, wiping sibling chains), so zero it once via DVE."""
            pv = ppv.tile([128, 260], f32, tag="pv", name="pv_ps")
            nc.vector.memset(pv, 0.0)
            return pv

        def emit_pv_mm(prev, pv, c2, c, s_sub, h):
            hidx = 2 * prev.p + h
            nc.tensor.matmul(
                pv[:, c2 * 130 + h * 65:c2 * 130 + h * 65 + 65],
                prev.exs[s_sub][:, h * 512 + c * 128:h * 512 + c * 128 + 128],
                vaug[s_sub][:, hidx * 65:hidx * 65 + 65],
                start=False,
                stop=(s_sub == 15),
                skip_group_check=True,
            )

        def emit_pv_slot(prev, st, smode=False):
            """PV for stage `prev` during slot st of the next stage.
            Steady state: t-subtiles 0,1 accumulate over s in slots 0-7;
            subtiles 2,3 in slots 8-15 (each slot: 2 chains x 2 s-tiles x
            2 heads = 8 MMs of 65 columns). smode (stage 1 only): all four
            chains advance one s-tile per slot, because vaug[s] is itself
            produced just-in-time by the V-projection during this stage."""
            if st == 0 and prev.pv[0] is None:
                prev.pv[0] = alloc_pv()
            if smode:
                if st == 0:
                    prev.pv[1] = alloc_pv()
                for c in range(4):
                    half, c2 = c // 2, c % 2
                    for h in range(2):
                        emit_pv_mm(prev, prev.pv[half], c2, c, st, h)
                return
            if st == 1 and prev.pv[1] is None:
                prev.pv[1] = alloc_pv()
            half, k = st // 8, st % 8
            pv = prev.pv[half]
            for c2 in range(2):
                c = 2 * half + c2
                for j in range(2):
                    s_sub = 2 * k + j
                    for h in range(2):
                        emit_pv_mm(prev, pv, c2, c, s_sub, h)

        def emit_normalize_half(prev, half):
            """Normalize the two finished chains of `half`: per head,
            reciprocal of the ones-column denominator (on partitions = t),
            then a per-partition scalar multiply -> OnormB [t,dc] bf16."""
            pv = prev.pv[half]
            for c2 in range(2):
                c = 2 * half + c2
                tt = prev.tq * 4 + c
                onb = small.tile([128, 128], bf16, tag="onb", name="onb", bufs=4)
                for h in range(2):
                    base = c2 * 130 + h * 65
                    rc = small.tile([128, 1], f32, tag="rc", name="rc")
                    nc.vector.reciprocal(rc, pv[:, base + 64:base + 65])
                    nc.vector.tensor_scalar_mul(
                        onb[:, h * 64:(h + 1) * 64],
                        pv[:, base:base + 64], rc)
                pending_tr.append((onb, prev.p, tt))
            prev.pv[half] = None

        def emit_flush_tr():
            """Transpose one pending OnormB block ([t,dc] -> [dc,t]) into
            its persistent SBUF slot via the DMA crossbar (SP queue): no PE
            or DVE time, and nothing for the engines to wait on."""
            if not pending_tr:
                return
            onb, p, tt = pending_tr.pop(0)
            nc.sync.dma_start_transpose(out=OnT[p][tt], in_=onb)

        def emit_stage(p, tq, prev, extras, smode=False):
            """16 score slots for (p, tq); interleave prev stage's PV and
            the extra thunks. `extras` is a list of (target_slot, thunk),
            non-decreasing in target: each thunk is emitted at the first
            slot >= its target (order preserved, so psum-ring windows of
            consecutive projection halves never interleave)."""
            t0 = tq * 512
            exs = []
            taken = 0
            my_pv0 = None
            for st in range(NS):
                sc_ps = psc.tile([128, 1024], f32, tag="sc", name="sc_ps")
                nc.tensor.matmul(
                    sc_ps[:, 0:512],
                    kTs[p][0:64, st * 128:(st + 1) * 128],
                    qTs[p][0:64, t0:t0 + 512],
                    start=True, stop=True,
                    tile_position=(0, 0),
                )
                nc.tensor.matmul(
                    sc_ps[:, 512:1024],
                    kTs[p][64:128, st * 128:(st + 1) * 128],
                    qTs[p][64:128, t0:t0 + 512],
                    start=True, stop=True,
                    tile_position=(64, 0),
                )
                ex = expool.tile([128, 1024], bf16, tag="ex", name="ex")
                nc.scalar.activation(ex, sc_ps, AF.Exp, scale=0.125)
                exs.append(ex)
                if prev is not None:
                    emit_pv_slot(prev, st, smode=smode)
                    if st == 7 and not smode:
                        emit_normalize_half(prev, 0)
                if st in (2, 3, 4, 9, 10, 11):
                    emit_flush_tr()
                if st == 9 and not smode:
                    # this stage's own half-0 accumulator, zeroed well before
                    # its first use at slot 0 of the next stage (the ring
                    # slot was freed by the slot-7 normalize two slots ago)
                    my_pv0 = alloc_pv()
                while taken < len(extras) and extras[taken][0] <= st:
                    extras[taken][1]()
                    taken += 1
            while taken < len(extras):
                extras[taken][1]()
                taken += 1
            if prev is not None:
                if smode:
                    emit_normalize_half(prev, 0)
                emit_normalize_half(prev, 1)
            return PrevStage(p, tq, exs, my_pv0)

        def tspread(thunks, lo, hi):
            """Evenly spaced slot targets in [lo, hi], order preserved."""
            n = len(thunks)
            if n <= 1:
                return [(lo, th) for th in thunks]
            return [(lo + (hi - lo) * i // (n - 1), th)
                    for i, th in enumerate(thunks)]

        def tmerge(*lists):
            out = [x for l in lists for x in l]
            out.sort(key=lambda x: x[0])  # stable: sublist order preserved
            return out

        # ---- emission ----
        for _rep in range(repeats):
            # startup: only pair-0's q/k half-0 projections block the first
            # scores. q streams on the SP DMA queue, k on the Activation
            # HWDGE queue; weight and x-chunk DMAs interleave per queue so
            # matmul e waits only for weight/chunk e, and the psums sit in
            # the (still idle) scores ring.
            wq_tiles, wk_tiles, q0x, k0x = [], [], [], []
            for e in range(8):
                wt = wpool.tile([128, DC], bf16, tag="w", name="wt")
                nc.sync.dma_start(out=wt, in_=WqT_d[e * 128:(e + 1) * 128, :])
                wq_tiles.append(wt)
                xt = xin.tile([128, 1024], bf16, tag="xq", name="xin", bufs=6)
                nc.sync.dma_start(out=xt, in_=qT_d[e * 128:(e + 1) * 128, 0:1024])
                q0x.append(xt)
                wt = wpool.tile([128, DC], bf16, tag="w", name="wt")
                nc.gpsimd.dma_start(out=wt, in_=WkT_d[e * 128:(e + 1) * 128, :])
                wk_tiles.append(wt)
                xt = xin.tile([128, 1024], bf16, tag="xk", name="xin", bufs=6)
                nc.gpsimd.dma_start(out=xt, in_=kT_d[e * 128:(e + 1) * 128, 0:1024])
                k0x.append(xt)
            wv_tiles = load_wtiles(WvT_d, nc.gpsimd)
            load_consts()
            ps_q = psc.tile([128, 1024], f32, tag="sc", name="pj_ps")
            ps_k = psc.tile([128, 1024], f32, tag="sc", name="pj_ps")
            for e in range(8):
                for ps, w_t, x_t in ((ps_q, wq_tiles, q0x), (ps_k, wk_tiles, k0x)):
                    for qi in range(2):
                        nc.tensor.matmul(
                            ps[:, qi * 512:(qi + 1) * 512],
                            w_t[e][:, 0:128],
                            x_t[e][:, qi * 512:(qi + 1) * 512],
                            start=(e == 0),
                            stop=(e == 7),
                        )
            for qi in range(2):
                nc.vector.tensor_scalar_add(
                    qTs[0][:, qi * 512:(qi + 1) * 512],
                    ps_q[:, qi * 512:(qi + 1) * 512], bq_sb[:, 0:1])
                nc.vector.tensor_scalar_add(
                    kTs[0][:, qi * 512:(qi + 1) * 512],
                    ps_k[:, qi * 512:(qi + 1) * 512], bk_sb[:, 0:1])

            q0h1 = proj_thunks(0, qT_d, wq_tiles, qTs[0], bq_sb, nc.sync,
                               "xq", halves=(1,))
            k0h1 = proj_thunks(0, kT_d, wk_tiles, kTs[0], bk_sb, nc.sync,
                               "xk", halves=(1,))
            vpA = vproj_thunks(wv_tiles, 0, (nc.sync, nc.sync))
            vpB = vproj_thunks(wv_tiles, 1, (nc.sync, nc.sync))
            kqp = {}
            for p in range(1, NP):
                kqp[p] = (proj_thunks(p, kT_d, wk_tiles, kTs[p], bk_sb,
                                      nc.sync, "xk"),
                          proj_thunks(p, qT_d, wq_tiles, qTs[p], bq_sb,
                                      nc.sync, "xq"))
            op0 = outproj_thunks(0)
            op1 = outproj_thunks(1)
            op2 = outproj_thunks(2)

            # per-stage (target_slot, thunk) schedules. Deadlines: k_p half0
            # before stage 4p, half1 before its slot 8; q_p half0 before
            # 4p, half1 before 4p+2; vaug quad A just-in-time for stage 1's
            # s-major PV, quad B before stage 9; out-proj units only after
            # their pair-3 OnT block is transposed (flush slots 9/10 same
            # stage, 2/3 next stage). Matching mx-psum users get disjoint
            # target windows so the 2-slot ring never cross-blocks.
            # v-projection lists are [load_h0, 16 half0 sub-thunks, load_h1,
            # 16 half1 sub-thunks]; out-proj lists are 16 sub-thunks per
            # quarter (two per (tt, c) unit)
            ex = {
                0: tmerge(tspread(k0h1, 0, 7), tspread(q0h1, 8, 14),
                          [(2, vpA[0])], tspread(vpA[1:17], 8, 15),
                          [(8, vpA[17])]),
                1: tmerge(tspread(vpA[18:34], 6, 13), [(0, vpB[0])]),
                2: tmerge(tspread(kqp[1][0][:10], 0, 8),
                          tspread(vpB[1:5], 9, 13)),
                3: tmerge(tspread(kqp[1][1][:10], 0, 8),
                          tspread(vpB[5:9], 9, 13)),
                4: tmerge(tspread(kqp[1][0][10:], 0, 7),
                          [(8, vpB[17])], tspread(vpB[9:13], 9, 13)),
                5: tmerge(tspread(kqp[1][1][10:], 0, 8),
                          tspread(vpB[13:17], 9, 13)),
                6: tmerge(tspread(kqp[2][0][:10], 0, 8),
                          tspread(vpB[18:22], 9, 13)),
                7: tmerge(tspread(kqp[2][1][:10], 0, 8),
                          tspread(vpB[22:26], 9, 13)),
                8: tmerge(tspread(kqp[2][0][10:], 0, 7),
                          tspread(vpB[26:34], 8, 15)),
                9: tmerge(tspread(kqp[2][1][10:], 0, 7),
                          tspread(kqp[3][0][:6], 8, 15)),
                10: tmerge(tspread(kqp[3][0][6:10], 0, 3),
                           tspread(kqp[3][1][:10], 4, 12)),
                11: tspread(kqp[3][0][10:], 0, 8),
                12: tspread(kqp[3][1][10:], 0, 8),
                13: tspread(op0[0:8], 11, 15),
                14: tmerge(tspread(op0[8:16], 4, 8), tspread(op1[0:8], 11, 15)),
                15: tmerge(tspread(op1[8:16], 4, 8), tspread(op2[0:8], 11, 15)),
            }

            prev = None
            for s in range(16):
                p, tq = s // 4, s % 4
                prev = emit_stage(p, tq, prev, ex.get(s, []), smode=(s == 1))

            # tail: PV of the last stage, out-proj(tt10..11) as soon as
            # their blocks flush, then the final normalize/transposes with
            # out-proj(t3) units covering the DVE latencies
            for st in range(NS):
                emit_pv_slot(prev, st)
                if st == 7:
                    emit_normalize_half(prev, 0)
                if st in (2, 3, 9, 10):
                    emit_flush_tr()
                if 4 <= st <= 7:
                    op2[8 + (st - 4) * 2]()
                    op2[8 + (st - 4) * 2 + 1]()
                if st == 11:
                    outproj_tail_unit(12)
            emit_normalize_half(prev, 1)
            emit_flush_tr()
            emit_flush_tr()
            outproj_tail_unit(13)
            outproj_tail_unit(14)
            outproj_tail_unit(15)

    nc.compile()
    return nc


def _get_nc():
    global _cached
    if _cached is None:
        _cached = _build()
    return _cached


def _prep_core_inputs(c, query, key, value, Wq, Wk, Wv, Wo, bq, bk, bv,
                      _cache={}):
    b, g = c // 2, c % 2
    sl = slice(g * DC, (g + 1) * DC)
    key_ = (id(query), b)
    if key_ not in _cache:
        # both cores of a batch share the transposed/cast activations
        _cache.clear()
        _cache[key_] = {
            "qT": query[b].T.astype(_BF16),
            "kT": key[b].T.astype(_BF16),
            "vT": value[b].T.astype(_BF16),
        }
    shared = _cache[key_]
    return {
        **shared,
        "WqT": Wq[sl].T.astype(_BF16),
        "WkT": Wk[sl].T.astype(_BF16),
        "WvT": Wv[sl].T.astype(_BF16),
        "WoS": Wo[:, sl].T.astype(_BF16),
        "bq": np.ascontiguousarray(bq[sl].reshape(NP, 128).T),
        "bk": np.ascontiguousarray(bk[sl].reshape(NP, 128).T),
        "bv": np.ascontiguousarray(bv[sl].reshape(1, DC)),
    }


def kernel(**inputs):
    from concourse.bass_utils import run_bass_kernel_spmd

    args = {k: np.asarray(inputs[k], np.float32)
            for k in ("query", "key", "value", "Wq", "Wk", "Wv", "Wo",
                      "bq", "bk", "bv", "bo")}
    _prep_core_inputs.__defaults__[0].clear()
    nc = _get_nc()
    in_maps = [
        _prep_core_inputs(c, args["query"], args["key"], args["value"],
                          args["Wq"], args["Wk"], args["Wv"], args["Wo"],
                          args["bq"], args["bk"], args["bv"])
        for c in range(8)
    ]
    res = run_bass_kernel_spmd(nc, in_maps, core_ids=list(range(8)))
    outs = [r["out"] for r in res.results]
    final = np.empty((B, T, E), np.float32)
    for b in range(B):
        final[b] = (outs[2 * b].astype(np.float32)
                    + outs[2 * b + 1].astype(np.float32)
                    + args["bo"][None, :])
    return final


# revision 35
# speedup vs baseline: 1.1545x; 1.1545x over previous
"""Multi-head attention (B=4, T=S=2048, E=1024, H=16, D=64) on 8 TRN2 NeuronCores.

Sharding: core c handles batch b=c//2 and head-group g=c%2 (8 of 16 heads).
Each core computes its 8 heads' attention plus the matching column-slice of
the output projection, producing a partial [T, E] bf16 output. Host sums the
two partials per batch and adds bo.

On-chip dataflow (all matmuls bf16 with fp32 PSUM accumulation):
  qT[d,t] = WqT.T @ queryT       (d-major projections, per 128-dim head pair)
  kT[d,t] likewise; v[s,d] natural via value.T as the stationary operand
  S.T[s,t] = kT_h.T @ qT_h       (two heads row-packed in the 128-row PE array)
  expS.T   = exp(S.T * 1/8)      (ScalarE, PSUM -> SBUF bf16)
  O[t,d],den = expS.T.T @ [v_h|1]  (t-major PV: full 128 output partitions,
                                    65-column moving operand -> half the PE
                                    column-cycles of the d-major layout)
  OnormB   = O * (1/den)         (DVE per-partition scalar multiply)
  OnormT   = transpose(OnormB)   (PE transpose via identity, [t,dc]->[dc,t])
  partial  = OnormT.T @ WoSlice  (accumulate over the core's 4 head pairs)

Emission is software-pipelined: stage s=(pair, t-quarter) in pair-major
order; each stage's 16 score-tile slots interleave the previous stage's PV
accumulation (t-subtiles 0,1 during slots 0-7; 2,3 during slots 8-15, so
normalization staggers) plus spread-out projection / v-projection /
out-projection work, keeping ScalarE (the exp bottleneck) continuously fed.
"""

from contextlib import ExitStack

import numpy as np
import ml_dtypes

B, T, S, E = 4, 2048, 2048, 1024
H, D = 16, 64
DC = 512          # dims per core (8 heads x 64)
NP = 4            # head pairs per core
NS = S // 128     # 16 s-tiles
NQ = 4            # t-quarters of 512

_BF16 = ml_dtypes.bfloat16

_cached = None


def _build(repeats=1):
    import concourse.bass as bass
    import concourse.mybir as mybir
    import concourse.tile as tile
    from concourse import bacc

    f32 = mybir.dt.float32
    bf16 = mybir.dt.bfloat16
    AF = mybir.ActivationFunctionType

    nc = bacc.Bacc("TRN2", target_bir_lowering=False)

    qT_d = nc.dram_tensor("qT", [E, T], bf16, kind="ExternalInput")
    kT_d = nc.dram_tensor("kT", [E, S], bf16, kind="ExternalInput")
    vT_d = nc.dram_tensor("vT", [E, S], bf16, kind="ExternalInput")
    WqT_d = nc.dram_tensor("WqT", [E, DC], bf16, kind="ExternalInput")
    WkT_d = nc.dram_tensor("WkT", [E, DC], bf16, kind="ExternalInput")
    WvT_d = nc.dram_tensor("WvT", [E, DC], bf16, kind="ExternalInput")
    WoS_d = nc.dram_tensor("WoS", [DC, E], bf16, kind="ExternalInput")
    bq_d = nc.dram_tensor("bq", [128, NP], f32, kind="ExternalInput")
    bk_d = nc.dram_tensor("bk", [128, NP], f32, kind="ExternalInput")
    bv_d = nc.dram_tensor("bv", [1, DC], f32, kind="ExternalInput")
    ident_d = nc.dram_tensor("ident", [128, 128], bf16, kind="ExternalInput")
    out_d = nc.dram_tensor("out", [T, E], bf16, kind="ExternalOutput")

    with tile.TileContext(nc) as tc, ExitStack() as ctx:
        persist = ctx.enter_context(tc.tile_pool(name="persist", bufs=1))
        psc = ctx.enter_context(tc.tile_pool(name="psc", bufs=2, space="PSUM"))
        ppv = ctx.enter_context(tc.tile_pool(name="ppv", bufs=2, space="PSUM"))
        pmx = ctx.enter_context(tc.tile_pool(name="pmx", bufs=2, space="PSUM"))
        expool = ctx.enter_context(tc.tile_pool(name="expool", bufs=26))
        small = ctx.enter_context(tc.tile_pool(name="small", bufs=3))
        ocp_pool = ctx.enter_context(tc.tile_pool(name="ocp", bufs=3))
        xin = ctx.enter_context(tc.tile_pool(name="xin", bufs=14))
        wpool = ctx.enter_context(tc.tile_pool(name="wts", bufs=24))

        # ---- persistent SBUF tiles ----
        qTs = [persist.tile([128, T], bf16, tag=f"qT{p}", name=f"qT{p}") for p in range(NP)]
        kTs = [persist.tile([128, S], bf16, tag=f"kT{p}", name=f"kT{p}") for p in range(NP)]
        vaug = [persist.tile([128, 8 * 65], bf16, tag=f"va{st}", name=f"va{st}") for st in range(NS)]
        WoSs = [persist.tile([128, E], bf16, tag=f"wo{p}", name=f"wo{p}") for p in range(NP)]
        # transposed normalized O blocks: OnT[p][tt] is [dc=128, t=128] bf16
        OnT = [[persist.tile([128, 128], bf16, tag=f"ot{p}_{t}", name=f"ot{p}_{t}")
                for t in range(16)] for p in range(NP)]
        bq_sb = persist.tile([128, NP], f32, tag="bq", name="bq_sb")
        bk_sb = persist.tile([128, NP], f32, tag="bk", name="bk_sb")
        bv_sb = persist.tile([128, DC], f32, tag="bv", name="bv_sb")
        ident_sb = persist.tile([128, 128], bf16, tag="id", name="ident_sb")

        # constants ride the (otherwise idle) GPSIMD software-DGE queue so
        # the two hardware queues stay clear for the streaming inputs; the
        # fatter ones (wv tiles, WoS, identity) are emitted inside the main
        # block after the v weights, ordered by first use
        nc.gpsimd.dma_start(out=bq_sb, in_=bq_d[:, :])
        nc.gpsimd.dma_start(out=bk_sb, in_=bk_d[:, :])

        def load_consts():
            bv_ap = bv_d[:, :]
            bv_bcast_ap = bass.AP(
                tensor=bv_ap.tensor,
                offset=bv_ap.offset,
                ap=[[0, 128], bv_ap.ap[-1]],
            )
            nc.gpsimd.dma_start(out=bv_sb, in_=bv_bcast_ap)
            nc.gpsimd.dma_start(out=ident_sb, in_=ident_d[:, :])
            for p in range(NP):
                nc.gpsimd.dma_start(out=WoSs[p], in_=WoS_d[p * 128:(p + 1) * 128, :])
        for st in range(NS):
            va3 = vaug[st].rearrange("p (h x) -> p h x", x=65)
            nc.vector.memset(va3[:, :, 64:65], 1.0)

        def load_wtiles(dram, deng):
            ts_ = []
            for e in range(8):
                t_ = wpool.tile([128, DC], bf16, tag="w", name="wt")
                deng.dma_start(out=t_, in_=dram[e * 128:(e + 1) * 128, :])
                ts_.append(t_)
            return ts_

        def proj_thunks(p, x_dram, w_tiles, dst, bias_sb, deng, xtag, halves=(0, 1),
                        preload=False, use_sc=False):
            """One pair's q/k projection as a thunk list: two column-halves;
            per half, stream 8 e-tile chunks (load + 2 quarter-MMs each),
            then bias-add the two finished quarters out of PSUM. With
            preload=True all 8 DMAs are issued before the first matmul;
            use_sc puts the psum in the (still idle) scores ring so the two
            startup halves can interleave without exhausting the mx slots."""
            thunks = []
            for half in halves:
                ps_pair = []  # the two quarter psums of this half (alloc lazily)
                xh = []

                def load_chunk(e, half, xh):
                    xt = xin.tile([128, 1024], bf16, tag=xtag,
                                  name="xin", bufs=6)
                    deng.dma_start(
                        out=xt,
                        in_=x_dram[e * 128:(e + 1) * 128,
                                   half * 1024:(half + 1) * 1024])
                    xh.append(xt)

                def open_half(half=half, ps_pair=ps_pair, xh=xh):
                    if use_sc:
                        ps = psc.tile([128, 1024], f32, tag="sc", name="pj_ps")
                        ps_pair.extend([ps[:, 0:512], ps[:, 512:1024]])
                    else:
                        for qi in range(2):
                            ps_pair.append(pmx.tile([128, 512], f32, tag="mx", name="mx_ps"))
                    # issue all DMAs up front (preload) or keep a 2-chunk
                    # prefetch lead so a matmul never waits the ~2.3us
                    # issue-to-data DMA latency
                    for e in range(8 if preload else 2):
                        load_chunk(e, half, xh)

                thunks.append(open_half)

                for e in range(8):
                    def echunk(e=e, half=half, ps_pair=ps_pair, xh=xh):
                        if not preload and e < 6:
                            load_chunk(e + 2, half, xh)
                        xt = xh[e]
                        for qi in range(2):
                            nc.tensor.matmul(
                                ps_pair[qi],
                                w_tiles[e][:, p * 128:(p + 1) * 128],
                                xt[:, qi * 512:(qi + 1) * 512],
                                start=(e == 0),
                                stop=(e == 7),
                            )
                    thunks.append(echunk)

                def close_half(half=half, ps_pair=ps_pair):
                    for qi in range(2):
                        q = half * 2 + qi
                        nc.vector.tensor_scalar_add(
                            dst[:, q * 512:(q + 1) * 512],
                            ps_pair[qi], bias_sb[:, p:p + 1])
                    ps_pair.clear()

                thunks.append(close_half)
            return thunks

        def vproj_thunks(wv_tiles, dh, dengs):
            """V projection for head-quad dh (4 heads, N=256), streamed in
            two s-halves. dh=0 feeds pairs 0-1 (needed by stage 1); dh=1
            feeds pairs 2-3 (needed from stage 9) and can spread late."""
            thunks = []
            for half in range(2):
                vh = []

                def load_half(half=half, vh=vh):
                    for e in range(8):
                        vt = xin.tile([128, 1024], bf16, tag="xv",
                                      name="xin", bufs=10)
                        dengs[half].dma_start(
                            out=vt,
                            in_=vT_d[e * 128:(e + 1) * 128,
                                     half * 1024:(half + 1) * 1024])
                        vh.append(vt)

                thunks.append(load_half)
                for sti in range(8):
                    holder = []

                    def vst_a(sti=sti, half=half, vh=vh, holder=holder):
                        # the "pv" slots now belong to the chain accumulators
                        # (strict ring order); stage 0's units ride the "sc"
                        # ring (scores psum has slack there), the rest "mx"
                        if dh == 0 and half == 0:
                            ps = psc.tile([128, 512], f32, tag="sc", name="vp_ps")
                        else:
                            ps = pmx.tile([128, 512], f32, tag="mx", name="mx_ps")
                        holder.append(ps)
                        for e in range(4):
                            nc.tensor.matmul(
                                ps[:, 0:256],
                                vh[e][:, sti * 128:(sti + 1) * 128],
                                wv_tiles[e][:, dh * 256:(dh + 1) * 256],
                                start=(e == 0),
                                stop=False,
                            )

                    def vst_b(sti=sti, half=half, vh=vh, holder=holder):
                        st = half * 8 + sti
                        ps = holder.pop()
                        for e in range(4, 8):
                            nc.tensor.matmul(
                                ps[:, 0:256],
                                vh[e][:, sti * 128:(sti + 1) * 128],
                                wv_tiles[e][:, dh * 256:(dh + 1) * 256],
                                start=False,
                                stop=(e == 7),
                            )
                        va3 = vaug[st].rearrange("p (h x) -> p h x", x=65)
                        nc.vector.tensor_add(
                            va3[:, dh * 4:(dh + 1) * 4, 0:64],
                            ps[:, 0:256].rearrange("p (h x) -> p h x", x=64),
                            bv_sb[:, dh * 256:(dh + 1) * 256].rearrange(
                                "p (h x) -> p h x", x=64),
                        )
                        if half == 0 and sti == 7:
                            vh.clear()
                    thunks.append(vst_a)
                    thunks.append(vst_b)
            return thunks

        def outproj_thunks(tq):
            """Each (tt, c) unit as two ~1k-cycle thunks (pairs 0-1 matmuls,
            then pairs 2-3 + copy-out) so the per-slot PE load stays smooth;
            output DMAs alternate between the two hardware queues."""
            thunks = []
            for ui, (tt, c) in enumerate([(tt, c)
                                          for tt in range(tq * 4, tq * 4 + 4)
                                          for c in range(2)]):
                holder = []

                def unit_a(tt=tt, c=c, holder=holder):
                    op_ps = pmx.tile([128, 512], f32, tag="mx", name="mx_ps")
                    holder.append(op_ps)
                    for p in range(2):
                        nc.tensor.matmul(
                            op_ps,
                            OnT[p][tt],
                            WoSs[p][:, c * 512:(c + 1) * 512],
                            start=(p == 0),
                            stop=False,
                        )

                def unit_b(tt=tt, c=c, holder=holder,
                           deng=nc.sync):
                    op_ps = holder.pop()
                    for p in range(2, NP):
                        nc.tensor.matmul(
                            op_ps,
                            OnT[p][tt],
                            WoSs[p][:, c * 512:(c + 1) * 512],
                            start=False,
                            stop=(p == 3),
                        )
                    oc = ocp_pool.tile([128, 512], bf16, tag="ocp", name="oc")
                    nc.vector.tensor_copy(oc, op_ps)
                    deng.dma_start(
                        out=out_d[tt * 128:(tt + 1) * 128,
                                  c * 512:(c + 1) * 512],
                        in_=oc)
                thunks.append(unit_a)
                thunks.append(unit_b)
            return thunks

        def outproj_tail_unit(tt):
            # tail variant: "sc" psum tiles are free once scoring has ended,
            # so use wide [128,1024] units to avoid mx-slot serialization
            op_ps = psc.tile([128, 1024], f32, tag="sc", name="sc_ps")
            for c in range(2):
                for p in range(NP):
                    nc.tensor.matmul(
                        op_ps[:, c * 512:(c + 1) * 512],
                        OnT[p][tt],
                        WoSs[p][:, c * 512:(c + 1) * 512],
                        start=(p == 0),
                        stop=(p == 3),
                    )
            oc = ocp_pool.tile([128, 1024], bf16, tag="ocpw", name="ocw")
            nc.vector.tensor_copy(oc, op_ps)
            (nc.gpsimd if tt >= 14 else nc.sync).dma_start(
                out=out_d[tt * 128:(tt + 1) * 128, :], in_=oc)

        class PrevStage:
            def __init__(self, p, tq, exs, pv0):
                self.p, self.tq, self.exs = p, tq, exs
                self.pv = [pv0, None]  # [128,260] psums: halves (tt 0,1 | 2,3)

        # (onb_tile, p, tt) records awaiting PE transpose + copy-out
        pending_tr = []

        def alloc_pv():
            """Chain accumulator: four start-less accumulation groups share
            the bank (a matmul `start` would pending-zero the whole 2KB
            region, wiping sibling chains), so zero it once via DVE."""
            pv = ppv.tile([128, 260], f32, tag="pv", name="pv_ps")
            nc.vector.memset(pv, 0.0)
            return pv

        def emit_pv_mm(prev, pv, c2, c, s_sub, h):
            hidx = 2 * prev.p + h
            nc.tensor.matmul(
                pv[:, c2 * 130 + h * 65:c2 * 130 + h * 65 + 65],
                prev.exs[s_sub][:, h * 512 + c * 128:h * 512 + c * 128 + 128],
                vaug[s_sub][:, hidx * 65:hidx * 65 + 65],
                start=False,
                stop=(s_sub == 15),
                skip_group_check=True,
            )

        def emit_pv_slot(prev, st, smode=False):
            """PV for stage `prev` during slot st of the next stage.
            Steady state: t-subtiles 0,1 accumulate over s in slots 0-7;
            subtiles 2,3 in slots 8-15 (each slot: 2 chains x 2 s-tiles x
            2 heads = 8 MMs of 65 columns). smode (stage 1 only): all four
            chains advance one s-tile per slot, because vaug[s] is itself
            produced just-in-time by the V-projection during this stage."""
            if st == 0 and prev.pv[0] is None:
                prev.pv[0] = alloc_pv()
            if smode:
                if st == 0:
                    prev.pv[1] = alloc_pv()
                for c in range(4):
                    half, c2 = c // 2, c % 2
                    for h in range(2):
                        emit_pv_mm(prev, prev.pv[half], c2, c, st, h)
                return
            if st == 1 and prev.pv[1] is None:
                prev.pv[1] = alloc_pv()
            half, k = st // 8, st % 8
            pv = prev.pv[half]
            for c2 in range(2):
                c = 2 * half + c2
                for j in range(2):
                    s_sub = 2 * k + j
                    for h in range(2):
                        emit_pv_mm(prev, pv, c2, c, s_sub, h)

        def emit_normalize_half(prev, half):
            """Normalize the two finished chains of `half`: per head,
            reciprocal of the ones-column denominator (on partitions = t),
            then a per-partition scalar multiply -> OnormB [t,dc] bf16."""
            pv = prev.pv[half]
            for c2 in range(2):
                c = 2 * half + c2
                tt = prev.tq * 4 + c
                onb = small.tile([128, 128], bf16, tag="onb", name="onb", bufs=4)
                for h in range(2):
                    base = c2 * 130 + h * 65
                    rc = small.tile([128, 1], f32, tag="rc", name="rc")
                    nc.vector.reciprocal(rc, pv[:, base + 64:base + 65])
                    nc.vector.tensor_scalar_mul(
                        onb[:, h * 64:(h + 1) * 64],
                        pv[:, base:base + 64], rc)
                pending_tr.append((onb, prev.p, tt))
            prev.pv[half] = None

        def emit_flush_tr():
            """Transpose one pending OnormB block ([t,dc] -> [dc,t]) on the
            PE (rides the sc psum ring) and copy it out to its persistent
            SBUF slot. Deferred a few slots after normalize so the PE never
            waits on the DVE."""
            if not pending_tr:
                return
            onb, p, tt = pending_tr.pop(0)
            tr = psc.tile([128, 128], bf16, tag="sc", name="tr_ps")
            nc.tensor.transpose(tr, onb, ident_sb)
            nc.vector.tensor_copy(OnT[p][tt], tr)

        def emit_stage(p, tq, prev, extras, smode=False):
            """16 score slots for (p, tq); interleave prev stage's PV and
            the extra thunks. `extras` is a list of (target_slot, thunk),
            non-decreasing in target: each thunk is emitted at the first
            slot >= its target (order preserved, so psum-ring windows of
            consecutive projection halves never interleave)."""
            t0 = tq * 512
            exs = []
            taken = 0
            my_pv0 = None
            for st in range(NS):
                sc_ps = psc.tile([128, 1024], f32, tag="sc", name="sc_ps")
                nc.tensor.matmul(
                    sc_ps[:, 0:512],
                    kTs[p][0:64, st * 128:(st + 1) * 128],
                    qTs[p][0:64, t0:t0 + 512],
                    start=True, stop=True,
                    tile_position=(0, 0),
                )
                nc.tensor.matmul(
                    sc_ps[:, 512:1024],
                    kTs[p][64:128, st * 128:(st + 1) * 128],
                    qTs[p][64:128, t0:t0 + 512],
                    start=True, stop=True,
                    tile_position=(64, 0),
                )
                ex = expool.tile([128, 1024], bf16, tag="ex", name="ex")
                nc.scalar.activation(ex, sc_ps, AF.Exp, scale=0.125)
                exs.append(ex)
                if prev is not None:
                    emit_pv_slot(prev, st, smode=smode)
                    if st == 7 and not smode:
                        emit_normalize_half(prev, 0)
                if st in (2, 3, 4, 9, 10, 11):
                    emit_flush_tr()
                if st == 9 and not smode:
                    # this stage's own half-0 accumulator, zeroed well before
                    # its first use at slot 0 of the next stage (the ring
                    # slot was freed by the slot-7 normalize two slots ago)
                    my_pv0 = alloc_pv()
                while taken < len(extras) and extras[taken][0] <= st:
                    extras[taken][1]()
                    taken += 1
            while taken < len(extras):
                extras[taken][1]()
                taken += 1
            if prev is not None:
                if smode:
                    emit_normalize_half(prev, 0)
                emit_normalize_half(prev, 1)
            return PrevStage(p, tq, exs, my_pv0)

        def tspread(thunks, lo, hi):
            """Evenly spaced slot targets in [lo, hi], order preserved."""
            n = len(thunks)
            if n <= 1:
                return [(lo, th) for th in thunks]
            return [(lo + (hi - lo) * i // (n - 1), th)
                    for i, th in enumerate(thunks)]

        def tmerge(*lists):
            out = [x for l in lists for x in l]
            out.sort(key=lambda x: x[0])  # stable: sublist order preserved
            return out

        # ---- emission ----
        for _rep in range(repeats):
            # startup: only pair-0's q/k half-0 projections block the first
            # scores. q streams on the SP DMA queue, k on the Activation
            # HWDGE queue; weight and x-chunk DMAs interleave per queue so
            # matmul e waits only for weight/chunk e, and the psums sit in
            # the (still idle) scores ring.
            wq_tiles, wk_tiles, q0x, k0x = [], [], [], []
            for e in range(8):
                wt = wpool.tile([128, DC], bf16, tag="w", name="wt")
                nc.sync.dma_start(out=wt, in_=WqT_d[e * 128:(e + 1) * 128, :])
                wq_tiles.append(wt)
                xt = xin.tile([128, 1024], bf16, tag="xq", name="xin", bufs=6)
                nc.sync.dma_start(out=xt, in_=qT_d[e * 128:(e + 1) * 128, 0:1024])
                q0x.append(xt)
                wt = wpool.tile([128, DC], bf16, tag="w", name="wt")
                nc.gpsimd.dma_start(out=wt, in_=WkT_d[e * 128:(e + 1) * 128, :])
                wk_tiles.append(wt)
                xt = xin.tile([128, 1024], bf16, tag="xk", name="xin", bufs=6)
                nc.gpsimd.dma_start(out=xt, in_=kT_d[e * 128:(e + 1) * 128, 0:1024])
                k0x.append(xt)
            wv_tiles = load_wtiles(WvT_d, nc.gpsimd)
            load_consts()
            ps_q = psc.tile([128, 1024], f32, tag="sc", name="pj_ps")
            ps_k = psc.tile([128, 1024], f32, tag="sc", name="pj_ps")
            for e in range(8):
                for ps, w_t, x_t in ((ps_q, wq_tiles, q0x), (ps_k, wk_tiles, k0x)):
                    for qi in range(2):
                        nc.tensor.matmul(
                            ps[:, qi * 512:(qi + 1) * 512],
                            w_t[e][:, 0:128],
                            x_t[e][:, qi * 512:(qi + 1) * 512],
                            start=(e == 0),
                            stop=(e == 7),
                        )
            for qi in range(2):
                nc.vector.tensor_scalar_add(
                    qTs[0][:, qi * 512:(qi + 1) * 512],
                    ps_q[:, qi * 512:(qi + 1) * 512], bq_sb[:, 0:1])
                nc.vector.tensor_scalar_add(
                    kTs[0][:, qi * 512:(qi + 1) * 512],
                    ps_k[:, qi * 512:(qi + 1) * 512], bk_sb[:, 0:1])

            q0h1 = proj_thunks(0, qT_d, wq_tiles, qTs[0], bq_sb, nc.sync,
                               "xq", halves=(1,))
            k0h1 = proj_thunks(0, kT_d, wk_tiles, kTs[0], bk_sb, nc.sync,
                               "xk", halves=(1,))
            vpA = vproj_thunks(wv_tiles, 0, (nc.sync, nc.sync))
            vpB = vproj_thunks(wv_tiles, 1, (nc.sync, nc.sync))
            kqp = {}
            for p in range(1, NP):
                kqp[p] = (proj_thunks(p, kT_d, wk_tiles, kTs[p], bk_sb,
                                      nc.sync, "xk"),
                          proj_thunks(p, qT_d, wq_tiles, qTs[p], bq_sb,
                                      nc.sync, "xq"))
            op0 = outproj_thunks(0)
            op1 = outproj_thunks(1)
            op2 = outproj_thunks(2)

            # per-stage (target_slot, thunk) schedules. Deadlines: k_p half0
            # before stage 4p, half1 before its slot 8; q_p half0 before
            # 4p, half1 before 4p+2; vaug quad A just-in-time for stage 1's
            # s-major PV, quad B before stage 9; out-proj units only after
            # their pair-3 OnT block is transposed (flush slots 9/10 same
            # stage, 2/3 next stage). Matching mx-psum users get disjoint
            # target windows so the 2-slot ring never cross-blocks.
            # v-projection lists are [load_h0, 16 half0 sub-thunks, load_h1,
            # 16 half1 sub-thunks]; out-proj lists are 16 sub-thunks per
            # quarter (two per (tt, c) unit)
            ex = {
                0: tmerge(tspread(k0h1, 0, 7), tspread(q0h1, 8, 14),
                          [(2, vpA[0])], tspread(vpA[1:17], 8, 15),
                          [(8, vpA[17])]),
                1: tmerge(tspread(vpA[18:34], 6, 13), [(0, vpB[0])]),
                2: tmerge(tspread(kqp[1][0][:10], 0, 8),
                          tspread(vpB[1:5], 9, 13)),
                3: tmerge(tspread(kqp[1][1][:10], 0, 8),
                          tspread(vpB[5:9], 9, 13)),
                4: tmerge(tspread(kqp[1][0][10:], 0, 7),
                          [(8, vpB[17])], tspread(vpB[9:13], 9, 13)),
                5: tmerge(tspread(kqp[1][1][10:], 0, 8),
                          tspread(vpB[13:17], 9, 13)),
                6: tmerge(tspread(kqp[2][0][:10], 0, 8),
                          tspread(vpB[18:22], 9, 13)),
                7: tmerge(tspread(kqp[2][1][:10], 0, 8),
                          tspread(vpB[22:26], 9, 13)),
                8: tmerge(tspread(kqp[2][0][10:], 0, 7),
                          tspread(vpB[26:34], 8, 15)),
                9: tmerge(tspread(kqp[2][1][10:], 0, 7),
                          tspread(kqp[3][0][:6], 8, 15)),
                10: tmerge(tspread(kqp[3][0][6:10], 0, 3),
                           tspread(kqp[3][1][:10], 4, 12)),
                11: tspread(kqp[3][0][10:], 0, 8),
                12: tspread(kqp[3][1][10:], 0, 8),
                13: tspread(op0[0:8], 11, 15),
                14: tmerge(tspread(op0[8:16], 4, 8), tspread(op1[0:8], 11, 15)),
                15: tmerge(tspread(op1[8:16], 4, 8), tspread(op2[0:8], 11, 15)),
            }

            prev = None
            for s in range(16):
                p, tq = s // 4, s % 4
                prev = emit_stage(p, tq, prev, ex.get(s, []), smode=(s == 1))

            # tail: PV of the last stage, out-proj(tt10..11) as soon as
            # their blocks flush, then the final normalize/transposes with
            # out-proj(t3) units covering the DVE latencies
            for st in range(NS):
                emit_pv_slot(prev, st)
                if st == 7:
                    emit_normalize_half(prev, 0)
                if st in (2, 3, 9, 10):
                    emit_flush_tr()
                if 4 <= st <= 7:
                    op2[8 + (st - 4) * 2]()
                    op2[8 + (st - 4) * 2 + 1]()
                if st == 11:
                    outproj_tail_unit(12)
            emit_normalize_half(prev, 1)
            outproj_tail_unit(13)
            emit_flush_tr()
            emit_flush_tr()
            outproj_tail_unit(14)
            outproj_tail_unit(15)

    nc.compile()
    return nc


def _get_nc():
    global _cached
    if _cached is None:
        _cached = _build()
    return _cached


def _prep_core_inputs(c, query, key, value, Wq, Wk, Wv, Wo, bq, bk, bv,
                      _cache={}):
    b, g = c // 2, c % 2
    sl = slice(g * DC, (g + 1) * DC)
    key_ = (id(query), b)
    if key_ not in _cache:
        # both cores of a batch share the transposed/cast activations
        _cache.clear()
        _cache[key_] = {
            "qT": query[b].T.astype(_BF16),
            "kT": key[b].T.astype(_BF16),
            "vT": value[b].T.astype(_BF16),
        }
    shared = _cache[key_]
    return {
        **shared,
        "WqT": Wq[sl].T.astype(_BF16),
        "WkT": Wk[sl].T.astype(_BF16),
        "WvT": Wv[sl].T.astype(_BF16),
        "WoS": Wo[:, sl].T.astype(_BF16),
        "bq": np.ascontiguousarray(bq[sl].reshape(NP, 128).T),
        "bk": np.ascontiguousarray(bk[sl].reshape(NP, 128).T),
        "bv": np.ascontiguousarray(bv[sl].reshape(1, DC)),
        "ident": np.eye(128, dtype=_BF16),
    }


def kernel(**inputs):
    from concourse.bass_utils import run_bass_kernel_spmd

    args = {k: np.asarray(inputs[k], np.float32)
            for k in ("query", "key", "value", "Wq", "Wk", "Wv", "Wo",
                      "bq", "bk", "bv", "bo")}
    _prep_core_inputs.__defaults__[0].clear()
    nc = _get_nc()
    in_maps = [
        _prep_core_inputs(c, args["query"], args["key"], args["value"],
                          args["Wq"], args["Wk"], args["Wv"], args["Wo"],
                          args["bq"], args["bk"], args["bv"])
        for c in range(8)
    ]
    res = run_bass_kernel_spmd(nc, in_maps, core_ids=list(range(8)))
    outs = [r["out"] for r in res.results]
    final = np.empty((B, T, E), np.float32)
    for b in range(B):
        final[b] = (outs[2 * b].astype(np.float32)
                    + outs[2 * b + 1].astype(np.float32)
                    + args["bo"][None, :])
    return final
